# revision 5
# baseline (speedup 1.0000x reference)
"""Trainium2 Bass kernel for BbRelProjection (per-sample QP projections).

Data-parallel over the batch: each of the 8 NeuronCores processes a
contiguous block of 524288 samples.  Per core the data is tiled as
[128 partitions, W samples, C components] with interleaved component
layout (matches DRAM row-major), and all per-sample math runs as
min/max/add chains on strided SBUF access patterns:

  QP1: y0 = clip(p0, lx, ux)
  QP2: q1 = min(p1, avg), q2 = max(p2, avg), avg = 0.5*(p1+p2),
       then clip to [lx, ux]   (equivalent to the where(swap, ...) form)
  QP3: t  = clip(max(pc, (pa+pb+pc)/3, 0.5*(pc+max(pa,pb))), ly, uy)
       ya = clip(pa, ly, t), yb = clip(pb, ly, t)
       (the nested where() in the reference collapses to this max chain)
"""

import numpy as np

import concourse.bass as bass
import concourse.bacc as bacc
import concourse.mybir as mybir
from concourse.tile import TileContext
from concourse import bass_utils

N_CORES = 8
BATCH = 4194304
PER_CORE = BATCH // N_CORES  # 524288
P = 128
W = 512  # samples per partition per tile
F32 = mybir.dt.float32

MAX = mybir.AluOpType.max
MIN = mybir.AluOpType.min
ADD = mybir.AluOpType.add
MULT = mybir.AluOpType.mult

ONE_THIRD = float(np.float32(1.0 / 3.0))


def build_bass(per_core: int = PER_CORE, w: int = W) -> bass.Bass:
    ntiles = per_core // (P * w)
    assert ntiles * P * w == per_core

    nc = bacc.Bacc()
    yp = nc.dram_tensor("y_pred", [per_core, 6], F32, kind="ExternalInput")
    cp = nc.dram_tensor("constr_para", [per_core, 4], F32, kind="ExternalInput")
    out = nc.dram_tensor("out", [per_core, 6], F32, kind="ExternalOutput")

    ypr = yp[:, :].rearrange("(n p w) c -> n p w c", p=P, w=w)
    cpr = cp[:, :].rearrange("(n p w) c -> n p w c", p=P, w=w)
    outr = out[:, :].rearrange("(n p w) c -> n p w c", p=P, w=w)

    with TileContext(nc) as tc:
        with (
            tc.tile_pool(name="io", bufs=3) as io_pool,
            tc.tile_pool(name="tmp", bufs=2) as tmp_pool,
        ):
            for i in range(ntiles):
                yt = io_pool.tile([P, w, 6], F32)
                ct = io_pool.tile([P, w, 4], F32)
                ot = io_pool.tile([P, w, 6], F32)
                nc.sync.dma_start(yt[:, :, :], ypr[i])
                nc.sync.dma_start(ct[:, :, :], cpr[i])

                A = tmp_pool.tile([P, w], F32)
                B = tmp_pool.tile([P, w], F32)
                C = tmp_pool.tile([P, w], F32)
                D = tmp_pool.tile([P, w], F32)
                E = tmp_pool.tile([P, w], F32)
                F = tmp_pool.tile([P, w], F32)

                lx = ct[:, :, 0]
                ux = ct[:, :, 1]
                ly = ct[:, :, 2]
                uy = ct[:, :, 3]
                p = [yt[:, :, c] for c in range(6)]
                o = [ot[:, :, c] for c in range(6)]

                V = nc.vector

                # --- QP1: clamp nose_x ---
                V.tensor_tensor(D, p[0], lx, MAX)
                V.tensor_tensor(o[0], D, ux, MIN)

                # --- QP2: 2-point isotonic + clip ---
                V.tensor_tensor(A, p[1], p[2], ADD)  # p1 + p2
                V.scalar_tensor_tensor(E, A, 0.5, p[1], MULT, MIN)  # q1
                V.scalar_tensor_tensor(F, A, 0.5, p[2], MULT, MAX)  # q2
                V.tensor_tensor(E, E, lx, MAX)
                V.tensor_tensor(o[1], E, ux, MIN)
                V.tensor_tensor(F, F, lx, MAX)
                V.tensor_tensor(o[2], F, ux, MIN)

                # --- QP3: t = clip(max(pc, t_all, t_one), ly, uy) ---
                V.tensor_tensor(B, p[3], p[4], ADD)       # pa + pb
                V.tensor_tensor(B, B, p[5], ADD)          # pa + pb + pc
                nc.scalar.mul(B, B, ONE_THIRD)            # t_all (on ACT)
                V.tensor_tensor(C, p[3], p[4], MAX)       # m
                V.tensor_tensor(C, C, p[5], ADD)          # pc + m
                V.scalar_tensor_tensor(B, C, 0.5, B, MULT, MAX)  # max(t_one, t_all)
                V.tensor_tensor(B, B, p[5], MAX)          # max(..., pc)
                V.tensor_tensor(B, B, ly, MAX)
                V.tensor_tensor(o[5], B, uy, MIN)         # t
                V.tensor_tensor(C, p[3], ly, MAX)
                V.tensor_tensor(o[3], C, o[5], MIN)       # ya
                V.tensor_tensor(D, p[4], ly, MAX)
                V.tensor_tensor(o[4], D, o[5], MIN)       # yb

                nc.scalar.dma_start(outr[i], ot[:, :, :])

    nc.finalize()
    return nc


_CACHE: dict = {}


def _get_nc() -> bass.Bass:
    if "nc" not in _CACHE:
        _CACHE["nc"] = build_bass()
    return _CACHE["nc"]


def run_sharded(y_pred: np.ndarray, constr_para: np.ndarray, **spmd_kwargs):
    """Shard over 8 cores, run, and return (full_output, BassKernelResults)."""
    nc = _get_nc()
    in_maps = [
        {
            "y_pred": y_pred[i * PER_CORE : (i + 1) * PER_CORE],
            "constr_para": constr_para[i * PER_CORE : (i + 1) * PER_CORE],
        }
        for i in range(N_CORES)
    ]
    res = bass_utils.run_bass_kernel_spmd(nc, in_maps, list(range(N_CORES)), **spmd_kwargs)
    full = np.concatenate([res.results[i]["out"] for i in range(N_CORES)], axis=0)
    return full, res


def kernel(y_pred: np.ndarray, constr_para: np.ndarray) -> np.ndarray:
    y_pred = np.ascontiguousarray(y_pred, dtype=np.float32)
    constr_para = np.ascontiguousarray(constr_para, dtype=np.float32)
    assert y_pred.shape == (BATCH, 6) and constr_para.shape == (BATCH, 4)
    full, _ = run_sharded(y_pred, constr_para)
    return full
